# revision 12
# baseline (speedup 1.0000x reference)
# Trainium2 Bass kernel for nn_EpipolarReward (retrieval_knn).
#
# Full computation per (batch x view-pair) p:
#   - L2-normalize desc_i (N,D) and desc_j (M,D), round to bf16
#   - gram g = di_n @ dj_n^T (bf16 x bf16 -> fp32 PSUM), both directions;
#     d2 = 2 - 2g (row/col ordering by g)
#   - top-2 per row and per column of g (DVE max8 over bf16 tiles)
#   - ratio test on squared distances: u0 < 0.64*u1 with u = relu(1 - g)
#   - mutual-nearest check + Sampson error on matched keypoints
# Sharding: P=120 pairs, embarrassingly parallel, 15 pairs per core on 8
# cores.  Each core outputs [err_sum, match_count]; the host sums partials
# and computes [exp(-mean), mean, count].
#
# v1.1: the sampson/accumulation tail runs on the Pool (GPSIMD) engine
# (arithmetic ops only; comparisons stay on DVE, which is the bottleneck
# engine) - frees ~2.5us/pair of DVE time at zero risk.
#
# Mutual matches are found transpose-free:
#   * both gram directions are computed with the same bf16 operands, so
#     g[i,j] (row direction) and gT[j,i] (col direction) are bit-identical
#     fp32 values, and their bf16 roundings agree.
#   * for each column j (partition layout j = t*128 + p) a one-pass
#     tensor_scalar builds OHc[j,:] = VJ[j] * (Cbf[j,:] == c0[j]) - for a
#     ratio-valid column the max is strictly separated so the one-hot is
#     exact with a single 1 at argmax.
#   * a small bf16 matmul scatters [1, c0, kx_hi, kx_lo, ky_hi, ky_lo] of
#     each valid column to its argmax row (fp32 PSUM sums).
#   * row i is a mutual match iff W[i]==1, scattered c0 == row max g0r[i]
#     (bit-exact), and VI[i].  A bf16 tie at the row max forces ratio==1 ->
#     VI=0, so ties cannot create false accepts.  W>=2 (two ratio-valid
#     columns argmaxing to the same row) is treated as no-match; with the
#     0.8 ratio test this is vanishingly rare.

import numpy as np

P_TOTAL, N, M, D = 120, 1024, 1024, 256
NCORES = 8
PPC = P_TOTAL // NCORES  # 15 pairs per core
NT = N // 128  # 8 row tiles
MT = M // 128  # 8 col tiles
DC = D // 128  # 2 contraction chunks

_CACHE = {}


def build_nc(ppc=PPC, repeat=1):
    import concourse.bass as bass
    import concourse.mybir as mybir
    from concourse import bacc
    from concourse.masks import make_identity
    from concourse.tile import TileContext

    f32 = mybir.dt.float32
    bf16 = mybir.dt.bfloat16
    u16 = mybir.dt.uint16
    Alu = mybir.AluOpType
    Act = mybir.ActivationFunctionType
    AX = mybir.AxisListType.X

    nc = bacc.Bacc(trn_type="TRN2", target_bir_lowering=False, debug=False,
                   num_devices=NCORES)

    di_d = nc.dram_tensor("desc_i", [ppc, N, D], f32, kind="ExternalInput")
    dj_d = nc.dram_tensor("desc_j", [ppc, M, D], f32, kind="ExternalInput")
    kpi_d = nc.dram_tensor("kp_i", [ppc, N, 2], f32, kind="ExternalInput")
    kpj_d = nc.dram_tensor("kp_j", [ppc, M, 2], f32, kind="ExternalInput")
    f_d = nc.dram_tensor("F_mat", [ppc, 3, 3], f32, kind="ExternalInput")
    out_d = nc.dram_tensor("out", [1, 2], f32, kind="ExternalOutput")

    with TileContext(nc) as tc:
        with (
            tc.tile_pool(name="const", bufs=1) as cpool,
            tc.tile_pool(name="desc", bufs=2) as dpool,
            tc.tile_pool(name="dt", bufs=2) as dtpool,
            tc.tile_pool(name="gbf", bufs=3) as gpool,
            tc.tile_pool(name="small", bufs=2) as spool,
            tc.tile_pool(name="tail", bufs=2) as tpool,
            tc.tile_pool(name="ohc", bufs=2) as ohpool,
            tc.tile_pool(name="pg", bufs=2, space="PSUM") as pgpool,
            tc.tile_pool(name="pt", bufs=1, space="PSUM") as ptpool,
            tc.tile_pool(name="pa", bufs=1, space="PSUM") as papool,
            tc.tile_pool(name="pat", bufs=1, space="PSUM") as patpool,
        ):
            # ---- constants ----
            ident = cpool.tile([128, 128], f32, tag="ident")
            make_identity(nc, ident)
            ident_bf = cpool.tile([128, 128], bf16, tag="ident_bf")
            nc.vector.tensor_copy(ident_bf, ident)
            ones_col = cpool.tile([128, 1], f32, tag="ones_col")
            nc.vector.memset(ones_col, 1.0)
            acc2 = cpool.tile([128, 2, 2], f32, tag="acc2")
            nc.vector.memset(acc2, 0.0)
            eps12 = cpool.tile([128, 1], f32, tag="eps12")
            nc.vector.memset(eps12, 1e-12)

            def load_normalize_transpose(desc_ap, p, tag):
                # load (128, NT, 256) with row index t*128+q -> partition q
                dsb = dpool.tile([128, NT, D], f32, tag=f"dsb_{tag}")
                nc.sync.dma_start(
                    out=dsb, in_=desc_ap[p].rearrange("(t q) d -> q t d", q=128))
                ss = spool.tile([128, NT], f32, tag=f"ss_{tag}")
                scr = spool.tile([128, NT, D], f32, tag=f"scr_{tag}",
                                 name=f"scr_{tag}")
                nc.gpsimd.tensor_mul(scr, dsb, dsb)
                nc.vector.reduce_sum(ss, scr, axis=AX)
                sq = spool.tile([128, NT], f32, tag=f"sq_{tag}")
                nc.scalar.activation(sq, ss, Act.Sqrt, bias=eps12)
                rs = spool.tile([128, NT], f32, tag=f"rs_{tag}")
                nc.vector.reciprocal(rs, sq)
                dnb = dpool.tile([128, NT, D], bf16, tag=f"dnb_{tag}")
                nc.gpsimd.tensor_mul(
                    dnb, dsb, rs.unsqueeze(2).to_broadcast([128, NT, D]))
                # transpose to (128d, DC, 1024n) bf16
                dt = dtpool.tile([128, DC, N], bf16, tag=f"dt_{tag}")
                for c in range(DC):
                    pt = ptpool.tile([128, 1024], bf16, tag="pt")
                    for t in range(NT):
                        nc.tensor.transpose(
                            pt[:, t * 128:(t + 1) * 128],
                            dnb[:, t, c * 128:(c + 1) * 128], ident_bf)
                    nc.scalar.copy(dt[:, c], pt)
                return dt

            def gram_scan(dta, dtb, tag, keep_tiles):
                # top-8 values per partition; optionally keep the bf16 tiles
                m8 = tpool.tile([128, NT, 8], bf16, tag=f"m8_{tag}",
                                name=f"m8_{tag}")
                tiles = []
                for t in range(NT):
                    gb = gpool.tile([128, M], bf16, tag=f"gb_{tag}",
                                    name=f"gb_{tag}",
                                    bufs=(NT + 1 if keep_tiles else 3))
                    pg = pgpool.tile([128, 1024], f32, tag="pg", name="pg")
                    for jc in range(2):
                        for c in range(DC):
                            nc.tensor.matmul(
                                pg[:, jc * 512:(jc + 1) * 512],
                                lhsT=dta[:, c, t * 128:(t + 1) * 128],
                                rhs=dtb[:, c, jc * 512:(jc + 1) * 512],
                                start=(c == 0), stop=(c == DC - 1))
                    nc.scalar.copy(gb, pg)
                    nc.vector.max(m8[:, t], gb)
                    if keep_tiles:
                        tiles.append(gb)
                return m8, tiles

            for p in range(ppc * repeat):
                p = p % ppc
                dti = load_normalize_transpose(di_d.ap(), p, "i")
                dtj = load_normalize_transpose(dj_d.ap(), p, "j")
                m8r, _ = gram_scan(dti, dtj, "r", keep_tiles=False)
                m8c, ctiles = gram_scan(dtj, dti, "c", keep_tiles=True)

                kpi = tpool.tile([128, NT, 2], f32, tag="kpi")
                nc.sync.dma_start(
                    out=kpi, in_=kpi_d.ap()[p].rearrange("(t q) c -> q t c", q=128))
                kpj = tpool.tile([128, MT, 2], f32, tag="kpj")
                nc.sync.dma_start(
                    out=kpj, in_=kpj_d.ap()[p].rearrange("(t q) c -> q t c", q=128))
                frow = tpool.tile([128, 9], f32, tag="frow")
                nc.sync.dma_start(
                    out=frow,
                    in_=f_d.ap()[p].rearrange("a b -> (a b)").partition_broadcast(128))

                # u = relu(1 - g) (prop. to d2) from top-2 row/col values
                dd = tpool.tile([128, 32], f32, tag="dd")
                nc.scalar.copy(dd[:, 0:8], m8r[:, :, 0])
                nc.scalar.copy(dd[:, 8:16], m8r[:, :, 1])
                nc.scalar.copy(dd[:, 16:24], m8c[:, :, 0])
                nc.scalar.copy(dd[:, 24:32], m8c[:, :, 1])
                uu = tpool.tile([128, 32], f32, tag="uu")
                nc.vector.tensor_scalar(uu, dd, -1.0, 1.0, op0=Alu.mult, op1=Alu.add)
                nc.vector.tensor_scalar_max(uu, uu, 0.0)
                # valid = u0 < 0.64*u1   (ratio(dist) < 0.8)
                vthr = tpool.tile([128, 16], f32, tag="vthr")
                nc.vector.tensor_scalar(vthr[:, 0:8], uu[:, 8:16], 0.64, None,
                                        op0=Alu.mult)
                nc.vector.tensor_scalar(vthr[:, 8:16], uu[:, 24:32], 0.64, None,
                                        op0=Alu.mult)
                vi = tpool.tile([128, 8], f32, tag="vi")
                nc.vector.tensor_tensor(vi, uu[:, 0:8], vthr[:, 0:8], Alu.is_lt)
                vj = tpool.tile([128, 8], f32, tag="vj")
                nc.vector.tensor_tensor(vj, uu[:, 16:24], vthr[:, 8:16], Alu.is_lt)

                # scatter table: [1, c0, kxh, kxl, kyh, kyl, 0, 0] per column
                ct = tpool.tile([128, MT, 8], bf16, tag="ct")
                nc.vector.memset(ct[:, :, 0], 1.0)
                nc.vector.tensor_copy(ct[:, :, 1], m8c[:, :, 0])
                tmh = tpool.tile([128, MT], f32, tag="tmh")
                tml = tpool.tile([128, MT], f32, tag="tml")
                for comp in range(2):
                    hi_c, lo_c = 2 + 2 * comp, 3 + 2 * comp
                    nc.vector.tensor_copy(ct[:, :, hi_c], kpj[:, :, comp])
                    nc.vector.tensor_copy(tmh, ct[:, :, hi_c])
                    nc.vector.tensor_sub(tml, kpj[:, :, comp], tmh)
                    nc.vector.tensor_copy(ct[:, :, lo_c], tml)
                nc.vector.memset(ct[:, :, 6], 0.0)
                nc.vector.memset(ct[:, :, 7], 0.0)

                pa = [papool.tile([8, 512], f32, tag=f"pa{ic}", name=f"pa{ic}")
                      for ic in range(2)]
                for t in range(MT):
                    # masked one-hot: VJ[j] * (Cbf[j,:] == c0[j])
                    oh = ohpool.tile([128, N], bf16, tag="oh", name="oh")
                    nc.vector.tensor_scalar(
                        oh, ctiles[t], dd[:, 16 + t:17 + t], vj[:, t:t + 1],
                        op0=Alu.is_equal, op1=Alu.mult)
                    for ic in range(2):
                        nc.tensor.matmul(
                            pa[ic], lhsT=ct[:, t],
                            rhs=oh[:, ic * 512:(ic + 1) * 512],
                            start=(t == 0), stop=(t == MT - 1))
                asb = tpool.tile([8, N], f32, tag="asb")
                for ic in range(2):
                    nc.scalar.copy(asb[:, ic * 512:(ic + 1) * 512], pa[ic])
                pat = patpool.tile([128, 64], f32, tag="pat")
                for t in range(NT):
                    nc.tensor.transpose(pat[:, t * 8:(t + 1) * 8],
                                        asb[:, t * 128:(t + 1) * 128], ident[:8, :8])
                at = tpool.tile([128, NT, 8], f32, tag="at")
                nc.scalar.copy(at.rearrange("p a b -> p (a b)"), pat)

                # mutual = vi & (W == 1) & (scattered c0 == row max)
                g0f = dd[:, 0:8]
                e1 = tpool.tile([128, 8], f32, tag="e1")
                nc.vector.tensor_scalar(e1, at[:, :, 0], 1.0, None, op0=Alu.is_equal)
                e2 = tpool.tile([128, 8], f32, tag="e2")
                nc.vector.tensor_tensor(e2, at[:, :, 1], g0f, Alu.is_equal)
                mu = tpool.tile([128, 8], f32, tag="mu")
                nc.gpsimd.tensor_mul(mu, vi, e1)
                nc.gpsimd.tensor_mul(mu, mu, e2)

                kx = tpool.tile([128, 8], f32, tag="kx")
                nc.gpsimd.tensor_add(kx, at[:, :, 2], at[:, :, 3])
                ky = tpool.tile([128, 8], f32, tag="ky")
                nc.gpsimd.tensor_add(ky, at[:, :, 4], at[:, :, 5])

                # sampson: xi = (kpi_x, kpi_y, 1), xj = (kx, ky, 1)
                # l_j[k] = F[k,0]*xi_x + F[k,1]*xi_y + F[k,2]
                # l_i[k] = F[0,k]*xj_x + F[1,k]*xj_y + F[2,k]
                ta = tpool.tile([128, 8], f32, tag="ta")
                tb = tpool.tile([128, 8], f32, tag="tb")
                lj = [tpool.tile([128, 8], f32, tag=f"lj{k}", name=f"lj{k}")
                      for k in range(3)]
                for k in range(3):
                    nc.gpsimd.tensor_scalar_mul(ta, kpi[:, :, 0],
                                                frow[:, 3 * k:3 * k + 1])
                    nc.gpsimd.tensor_scalar_mul(tb, kpi[:, :, 1],
                                                frow[:, 3 * k + 1:3 * k + 2])
                    nc.gpsimd.tensor_add(lj[k], ta, tb)
                    nc.gpsimd.tensor_scalar_add(lj[k], lj[k],
                                                frow[:, 3 * k + 2:3 * k + 3])
                li = [tpool.tile([128, 8], f32, tag=f"li{k}", name=f"li{k}")
                      for k in range(2)]
                for k in range(2):
                    nc.gpsimd.tensor_scalar_mul(ta, kx, frow[:, k:k + 1])
                    nc.gpsimd.tensor_scalar_mul(tb, ky, frow[:, 3 + k:4 + k])
                    nc.gpsimd.tensor_add(li[k], ta, tb)
                    nc.gpsimd.tensor_scalar_add(li[k], li[k], frow[:, 6 + k:7 + k])
                s = tpool.tile([128, 8], f32, tag="s")
                nc.gpsimd.tensor_mul(s, kx, lj[0])
                nc.gpsimd.tensor_mul(ta, ky, lj[1])
                nc.gpsimd.tensor_add(s, s, ta)
                nc.gpsimd.tensor_add(s, s, lj[2])
                num = tpool.tile([128, 8], f32, tag="num")
                nc.gpsimd.tensor_mul(num, s, s)
                den = tpool.tile([128, 8], f32, tag="den")
                nc.gpsimd.tensor_mul(den, lj[0], lj[0])
                nc.gpsimd.tensor_mul(ta, lj[1], lj[1])
                nc.gpsimd.tensor_add(den, den, ta)
                nc.gpsimd.tensor_mul(ta, li[0], li[0])
                nc.gpsimd.tensor_add(den, den, ta)
                nc.gpsimd.tensor_mul(ta, li[1], li[1])
                nc.gpsimd.tensor_add(den, den, ta)
                nc.gpsimd.tensor_scalar_add(den, den, 1e-6)
                rden = tpool.tile([128, 8], f32, tag="rden")
                nc.vector.reciprocal(rden, den)
                err = tpool.tile([128, 8], f32, tag="err")
                nc.gpsimd.tensor_mul(err, num, rden)
                nc.gpsimd.tensor_mul(err, err, mu)
                rsum = tpool.tile([128, 2], f32, tag="rsum")
                t4 = tpool.tile([128, 2, 4], f32, tag="t4")
                t2 = tpool.tile([128, 2, 2], f32, tag="t2")
                for k, srcx in enumerate((err, mu)):
                    nc.gpsimd.tensor_add(t4[:, k], srcx[:, 0:4], srcx[:, 4:8])
                    nc.gpsimd.tensor_add(t2[:, k], t4[:, k, 0:2], t4[:, k, 2:4])
                    nc.gpsimd.tensor_add(rsum[:, k:k + 1], t2[:, k, 0:1],
                                         t2[:, k, 1:2])
                nc.gpsimd.tensor_add(acc2[:, p % 2], acc2[:, p % 2], rsum)

            # final cross-partition reduce: out[0, :] = sum_p acc[p, :]
            acc = cpool.tile([128, 2], f32, tag="accf")
            nc.vector.tensor_add(acc, acc2[:, 0], acc2[:, 1])
            pf = papool.tile([1, 2], f32, tag="pa0")
            nc.tensor.matmul(pf, lhsT=ones_col, rhs=acc, start=True, stop=True)
            osb = cpool.tile([1, 2], f32, tag="osb")
            nc.vector.tensor_copy(osb, pf)
            nc.sync.dma_start(out=out_d.ap(), in_=osb)

    nc.compile()
    return nc


def get_nc(ppc=PPC):
    if ppc not in _CACHE:
        _CACHE[ppc] = build_nc(ppc)
    return _CACHE[ppc]


def kernel(**inputs):
    from concourse.bass_utils import run_bass_kernel_spmd

    nc = get_nc(PPC)
    names = ["desc_i", "desc_j", "kp_i", "kp_j", "F_mat"]
    arrs = {k: np.ascontiguousarray(np.asarray(inputs[k]), dtype=np.float32)
            for k in names}
    in_maps = []
    for c in range(NCORES):
        sl = slice(c * PPC, (c + 1) * PPC)
        in_maps.append({k: arrs[k][sl] for k in names})
    res = run_bass_kernel_spmd(nc, in_maps, core_ids=list(range(NCORES)))
    total_err = 0.0
    total_cnt = 0.0
    for c in range(NCORES):
        o = np.asarray(res.results[c]["out"]).reshape(-1)
        total_err += float(o[0])
        total_cnt += float(o[1])
    mean_err = np.float32(total_err) / np.float32(max(total_cnt, 1.0))
    reward = np.exp(np.float32(-mean_err))
    return np.stack([np.float32(reward), np.float32(mean_err),
                     np.float32(total_cnt)]).astype(np.float32)



# revision 13
# speedup vs baseline: 1.3218x; 1.3218x over previous
# Trainium2 Bass kernel for nn_EpipolarReward (retrieval_knn).
#
# Full computation per (batch x view-pair) p:
#   - L2-normalize desc_i (N,D) and desc_j (M,D), round to bf16
#   - gram g = di_n @ dj_n^T (bf16 x bf16 -> fp32 PSUM), both directions;
#     d2 = 2 - 2g (row/col ordering by g)
#   - top-2 per row and per column of g (DVE max8 over bf16 tiles)
#   - ratio test on squared distances: u0 < 0.64*u1 with u = relu(1 - g)
#   - mutual-nearest check + Sampson error on matched keypoints
# Sharding: P=120 pairs, embarrassingly parallel, 15 pairs per core on 8
# cores.  Each core outputs [err_sum, match_count]; the host sums partials
# and computes [exp(-mean), mean, count].
#
# v1.1: the sampson/accumulation tail runs on the Pool (GPSIMD) engine
# (arithmetic ops only; comparisons stay on DVE, which is the bottleneck
# engine) - frees ~2.5us/pair of DVE time at zero risk.
#
# Mutual matches are found transpose-free:
#   * both gram directions are computed with the same bf16 operands, so
#     g[i,j] (row direction) and gT[j,i] (col direction) are bit-identical
#     fp32 values, and their bf16 roundings agree.
#   * for each column j (partition layout j = t*128 + p) a one-pass
#     tensor_scalar builds OHc[j,:] = VJ[j] * (Cbf[j,:] == c0[j]) - for a
#     ratio-valid column the max is strictly separated so the one-hot is
#     exact with a single 1 at argmax.
#   * a small bf16 matmul scatters [1, c0, kx_hi, kx_lo, ky_hi, ky_lo] of
#     each valid column to its argmax row (fp32 PSUM sums).
#   * row i is a mutual match iff W[i]==1, scattered c0 == row max g0r[i]
#     (bit-exact), and VI[i].  A bf16 tie at the row max forces ratio==1 ->
#     VI=0, so ties cannot create false accepts.  W>=2 (two ratio-valid
#     columns argmaxing to the same row) is treated as no-match; with the
#     0.8 ratio test this is vanishingly rare.

import numpy as np

P_TOTAL, N, M, D = 120, 1024, 1024, 256
NCORES = 8
PPC = P_TOTAL // NCORES  # 15 pairs per core
NT = N // 128  # 8 row tiles
MT = M // 128  # 8 col tiles
DC = D // 128  # 2 contraction chunks

_CACHE = {}


def build_nc(ppc=PPC, repeat=1):
    import concourse.bass as bass
    import concourse.mybir as mybir
    from concourse import bacc
    from concourse.masks import make_identity
    from concourse.tile import TileContext

    f32 = mybir.dt.float32
    bf16 = mybir.dt.bfloat16
    u16 = mybir.dt.uint16
    Alu = mybir.AluOpType
    Act = mybir.ActivationFunctionType
    AX = mybir.AxisListType.X

    nc = bacc.Bacc(trn_type="TRN2", target_bir_lowering=False, debug=False,
                   num_devices=NCORES)

    di_d = nc.dram_tensor("desc_i", [ppc, N, D], f32, kind="ExternalInput")
    dj_d = nc.dram_tensor("desc_j", [ppc, M, D], f32, kind="ExternalInput")
    kpi_d = nc.dram_tensor("kp_i", [ppc, N, 2], f32, kind="ExternalInput")
    kpj_d = nc.dram_tensor("kp_j", [ppc, M, 2], f32, kind="ExternalInput")
    f_d = nc.dram_tensor("F_mat", [ppc, 3, 3], f32, kind="ExternalInput")
    out_d = nc.dram_tensor("out", [1, 2], f32, kind="ExternalOutput")

    with TileContext(nc) as tc:
        with (
            tc.tile_pool(name="const", bufs=1) as cpool,
            tc.tile_pool(name="desc", bufs=2) as dpool,
            tc.tile_pool(name="dt", bufs=2) as dtpool,
            tc.tile_pool(name="gbf", bufs=3) as gpool,
            tc.tile_pool(name="small", bufs=2) as spool,
            tc.tile_pool(name="tail", bufs=2) as tpool,
            tc.tile_pool(name="ohc", bufs=2) as ohpool,
            tc.tile_pool(name="pg", bufs=2, space="PSUM") as pgpool,
            tc.tile_pool(name="pt", bufs=1, space="PSUM") as ptpool,
            tc.tile_pool(name="pa", bufs=1, space="PSUM") as papool,
            tc.tile_pool(name="pat", bufs=1, space="PSUM") as patpool,
        ):
            # ---- constants ----
            ident = cpool.tile([128, 128], f32, tag="ident")
            make_identity(nc, ident)
            ident_bf = cpool.tile([128, 128], bf16, tag="ident_bf")
            nc.vector.tensor_copy(ident_bf, ident)
            ones_col = cpool.tile([128, 1], f32, tag="ones_col")
            nc.vector.memset(ones_col, 1.0)
            acc2 = cpool.tile([128, 2, 2], f32, tag="acc2")
            nc.vector.memset(acc2, 0.0)
            eps12 = cpool.tile([128, 1], f32, tag="eps12")
            nc.vector.memset(eps12, 1e-12)

            def norm_a(desc_ap, p, tag):
                # load + norm latency chain (squares -> reduce -> sqrt ->
                # 1/x); emitted one pair EARLY so the DVE reduce/reciprocal
                # never head-of-line block the match phase's max8 scans.
                dsb = dpool.tile([128, NT, D], f32, tag=f"dsb_{tag}")
                nc.sync.dma_start(
                    out=dsb, in_=desc_ap[p].rearrange("(t q) d -> q t d", q=128))
                ss = spool.tile([128, NT], f32, tag=f"ss_{tag}")
                scr = spool.tile([128, NT, D], f32, tag=f"scr_{tag}",
                                 name=f"scr_{tag}")
                nc.gpsimd.tensor_mul(scr, dsb, dsb)
                nc.vector.reduce_sum(ss, scr, axis=AX)
                sq = spool.tile([128, NT], f32, tag=f"sq_{tag}")
                nc.scalar.activation(sq, ss, Act.Sqrt, bias=eps12)
                rs = spool.tile([128, NT], f32, tag=f"rs_{tag}")
                nc.vector.reciprocal(rs, sq)
                return dsb, rs

            def norm_b(dsb, rs, tag):
                # scale + transpose, emitted in the original (pair-local)
                # slot so the PE/PSUM transpose traffic keeps its schedule.
                dnb = dpool.tile([128, NT, D], bf16, tag=f"dnb_{tag}")
                nc.gpsimd.tensor_mul(
                    dnb, dsb, rs.unsqueeze(2).to_broadcast([128, NT, D]))
                dt = dtpool.tile([128, DC, N], bf16, tag=f"dt_{tag}")
                for c in range(DC):
                    pt = ptpool.tile([128, 1024], bf16, tag="pt")
                    for t in range(NT):
                        nc.tensor.transpose(
                            pt[:, t * 128:(t + 1) * 128],
                            dnb[:, t, c * 128:(c + 1) * 128], ident_bf)
                    nc.scalar.copy(dt[:, c], pt)
                return dt

            def gram_scan(dta, dtb, tag, keep_tiles):
                # top-8 values per partition; optionally keep the bf16 tiles
                m8 = tpool.tile([128, NT, 8], bf16, tag=f"m8_{tag}",
                                name=f"m8_{tag}")
                tiles = []
                for t in range(NT):
                    gb = gpool.tile([128, M], bf16, tag=f"gb_{tag}",
                                    name=f"gb_{tag}",
                                    bufs=(NT + 1 if keep_tiles else 3))
                    pg = pgpool.tile([128, 1024], f32, tag="pg", name="pg")
                    for jc in range(2):
                        for c in range(DC):
                            nc.tensor.matmul(
                                pg[:, jc * 512:(jc + 1) * 512],
                                lhsT=dta[:, c, t * 128:(t + 1) * 128],
                                rhs=dtb[:, c, jc * 512:(jc + 1) * 512],
                                start=(c == 0), stop=(c == DC - 1))
                    nc.scalar.copy(gb, pg)
                    nc.vector.max(m8[:, t], gb)
                    if keep_tiles:
                        tiles.append(gb)
                return m8, tiles

            npairs = ppc * repeat
            na = (norm_a(di_d.ap(), 0, "i"), norm_a(dj_d.ap(), 0, "j"))
            for pi in range(npairs):
                p = pi % ppc
                dti = norm_b(*na[0], "i")
                dtj = norm_b(*na[1], "j")
                m8r, _ = gram_scan(dti, dtj, "r", keep_tiles=False)
                m8c, ctiles = gram_scan(dtj, dti, "c", keep_tiles=True)
                if pi + 1 < npairs:
                    pn = (pi + 1) % ppc
                    na = (norm_a(di_d.ap(), pn, "i"), norm_a(dj_d.ap(), pn, "j"))

                kpi = tpool.tile([128, NT, 2], f32, tag="kpi")
                nc.sync.dma_start(
                    out=kpi, in_=kpi_d.ap()[p].rearrange("(t q) c -> q t c", q=128))
                kpj = tpool.tile([128, MT, 2], f32, tag="kpj")
                nc.sync.dma_start(
                    out=kpj, in_=kpj_d.ap()[p].rearrange("(t q) c -> q t c", q=128))
                frow = tpool.tile([128, 9], f32, tag="frow")
                nc.sync.dma_start(
                    out=frow,
                    in_=f_d.ap()[p].rearrange("a b -> (a b)").partition_broadcast(128))

                # u = relu(1 - g) (prop. to d2) from top-2 row/col values
                dd = tpool.tile([128, 32], f32, tag="dd")
                nc.scalar.copy(dd[:, 0:8], m8r[:, :, 0])
                nc.scalar.copy(dd[:, 8:16], m8r[:, :, 1])
                nc.scalar.copy(dd[:, 16:24], m8c[:, :, 0])
                nc.scalar.copy(dd[:, 24:32], m8c[:, :, 1])
                uu = tpool.tile([128, 32], f32, tag="uu")
                nc.vector.tensor_scalar(uu, dd, -1.0, 1.0, op0=Alu.mult, op1=Alu.add)
                nc.vector.tensor_scalar_max(uu, uu, 0.0)
                # valid = u0 < 0.64*u1   (ratio(dist) < 0.8)
                vthr = tpool.tile([128, 16], f32, tag="vthr")
                nc.vector.tensor_scalar(vthr[:, 0:8], uu[:, 8:16], 0.64, None,
                                        op0=Alu.mult)
                nc.vector.tensor_scalar(vthr[:, 8:16], uu[:, 24:32], 0.64, None,
                                        op0=Alu.mult)
                vi = tpool.tile([128, 8], f32, tag="vi")
                nc.vector.tensor_tensor(vi, uu[:, 0:8], vthr[:, 0:8], Alu.is_lt)
                vj = tpool.tile([128, 8], f32, tag="vj")
                nc.vector.tensor_tensor(vj, uu[:, 16:24], vthr[:, 8:16], Alu.is_lt)

                # scatter table: [1, c0, kxh, kxl, kyh, kyl, 0, 0] per column
                ct = tpool.tile([128, MT, 8], bf16, tag="ct")
                nc.vector.memset(ct[:, :, 0], 1.0)
                nc.vector.tensor_copy(ct[:, :, 1], m8c[:, :, 0])
                tmh = tpool.tile([128, MT], f32, tag="tmh")
                tml = tpool.tile([128, MT], f32, tag="tml")
                for comp in range(2):
                    hi_c, lo_c = 2 + 2 * comp, 3 + 2 * comp
                    nc.vector.tensor_copy(ct[:, :, hi_c], kpj[:, :, comp])
                    nc.vector.tensor_copy(tmh, ct[:, :, hi_c])
                    nc.vector.tensor_sub(tml, kpj[:, :, comp], tmh)
                    nc.vector.tensor_copy(ct[:, :, lo_c], tml)
                nc.vector.memset(ct[:, :, 6], 0.0)
                nc.vector.memset(ct[:, :, 7], 0.0)

                pa = [papool.tile([8, 512], f32, tag=f"pa{ic}", name=f"pa{ic}")
                      for ic in range(2)]
                for t in range(MT):
                    # masked one-hot: VJ[j] * (Cbf[j,:] == c0[j])
                    oh = ohpool.tile([128, N], bf16, tag="oh", name="oh")
                    nc.vector.tensor_scalar(
                        oh, ctiles[t], dd[:, 16 + t:17 + t], vj[:, t:t + 1],
                        op0=Alu.is_equal, op1=Alu.mult)
                    for ic in range(2):
                        nc.tensor.matmul(
                            pa[ic], lhsT=ct[:, t],
                            rhs=oh[:, ic * 512:(ic + 1) * 512],
                            start=(t == 0), stop=(t == MT - 1))
                asb = tpool.tile([8, N], f32, tag="asb")
                for ic in range(2):
                    nc.scalar.copy(asb[:, ic * 512:(ic + 1) * 512], pa[ic])
                pat = patpool.tile([128, 64], f32, tag="pat")
                for t in range(NT):
                    nc.tensor.transpose(pat[:, t * 8:(t + 1) * 8],
                                        asb[:, t * 128:(t + 1) * 128], ident[:8, :8])
                at = tpool.tile([128, NT, 8], f32, tag="at")
                nc.scalar.copy(at.rearrange("p a b -> p (a b)"), pat)

                # mutual = vi & (W == 1) & (scattered c0 == row max)
                g0f = dd[:, 0:8]
                e1 = tpool.tile([128, 8], f32, tag="e1")
                nc.vector.tensor_scalar(e1, at[:, :, 0], 1.0, None, op0=Alu.is_equal)
                e2 = tpool.tile([128, 8], f32, tag="e2")
                nc.vector.tensor_tensor(e2, at[:, :, 1], g0f, Alu.is_equal)
                mu = tpool.tile([128, 8], f32, tag="mu")
                nc.gpsimd.tensor_mul(mu, vi, e1)
                nc.gpsimd.tensor_mul(mu, mu, e2)

                kx = tpool.tile([128, 8], f32, tag="kx")
                nc.gpsimd.tensor_add(kx, at[:, :, 2], at[:, :, 3])
                ky = tpool.tile([128, 8], f32, tag="ky")
                nc.gpsimd.tensor_add(ky, at[:, :, 4], at[:, :, 5])

                # sampson: xi = (kpi_x, kpi_y, 1), xj = (kx, ky, 1)
                # l_j[k] = F[k,0]*xi_x + F[k,1]*xi_y + F[k,2]
                # l_i[k] = F[0,k]*xj_x + F[1,k]*xj_y + F[2,k]
                ta = tpool.tile([128, 8], f32, tag="ta")
                tb = tpool.tile([128, 8], f32, tag="tb")
                lj = [tpool.tile([128, 8], f32, tag=f"lj{k}", name=f"lj{k}")
                      for k in range(3)]
                for k in range(3):
                    nc.gpsimd.tensor_scalar_mul(ta, kpi[:, :, 0],
                                                frow[:, 3 * k:3 * k + 1])
                    nc.gpsimd.tensor_scalar_mul(tb, kpi[:, :, 1],
                                                frow[:, 3 * k + 1:3 * k + 2])
                    nc.gpsimd.tensor_add(lj[k], ta, tb)
                    nc.gpsimd.tensor_scalar_add(lj[k], lj[k],
                                                frow[:, 3 * k + 2:3 * k + 3])
                li = [tpool.tile([128, 8], f32, tag=f"li{k}", name=f"li{k}")
                      for k in range(2)]
                for k in range(2):
                    nc.gpsimd.tensor_scalar_mul(ta, kx, frow[:, k:k + 1])
                    nc.gpsimd.tensor_scalar_mul(tb, ky, frow[:, 3 + k:4 + k])
                    nc.gpsimd.tensor_add(li[k], ta, tb)
                    nc.gpsimd.tensor_scalar_add(li[k], li[k], frow[:, 6 + k:7 + k])
                s = tpool.tile([128, 8], f32, tag="s")
                nc.gpsimd.tensor_mul(s, kx, lj[0])
                nc.gpsimd.tensor_mul(ta, ky, lj[1])
                nc.gpsimd.tensor_add(s, s, ta)
                nc.gpsimd.tensor_add(s, s, lj[2])
                num = tpool.tile([128, 8], f32, tag="num")
                nc.gpsimd.tensor_mul(num, s, s)
                den = tpool.tile([128, 8], f32, tag="den")
                nc.gpsimd.tensor_mul(den, lj[0], lj[0])
                nc.gpsimd.tensor_mul(ta, lj[1], lj[1])
                nc.gpsimd.tensor_add(den, den, ta)
                nc.gpsimd.tensor_mul(ta, li[0], li[0])
                nc.gpsimd.tensor_add(den, den, ta)
                nc.gpsimd.tensor_mul(ta, li[1], li[1])
                nc.gpsimd.tensor_add(den, den, ta)
                nc.gpsimd.tensor_scalar_add(den, den, 1e-6)
                rden = tpool.tile([128, 8], f32, tag="rden")
                nc.vector.reciprocal(rden, den)
                err = tpool.tile([128, 8], f32, tag="err")
                nc.gpsimd.tensor_mul(err, num, rden)
                nc.gpsimd.tensor_mul(err, err, mu)
                rsum = tpool.tile([128, 2], f32, tag="rsum")
                t4 = tpool.tile([128, 2, 4], f32, tag="t4")
                t2 = tpool.tile([128, 2, 2], f32, tag="t2")
                for k, srcx in enumerate((err, mu)):
                    nc.gpsimd.tensor_add(t4[:, k], srcx[:, 0:4], srcx[:, 4:8])
                    nc.gpsimd.tensor_add(t2[:, k], t4[:, k, 0:2], t4[:, k, 2:4])
                    nc.gpsimd.tensor_add(rsum[:, k:k + 1], t2[:, k, 0:1],
                                         t2[:, k, 1:2])
                nc.gpsimd.tensor_add(acc2[:, p % 2], acc2[:, p % 2], rsum)

            # final cross-partition reduce: out[0, :] = sum_p acc[p, :]
            acc = cpool.tile([128, 2], f32, tag="accf")
            nc.vector.tensor_add(acc, acc2[:, 0], acc2[:, 1])
            pf = papool.tile([1, 2], f32, tag="pa0")
            nc.tensor.matmul(pf, lhsT=ones_col, rhs=acc, start=True, stop=True)
            osb = cpool.tile([1, 2], f32, tag="osb")
            nc.vector.tensor_copy(osb, pf)
            nc.sync.dma_start(out=out_d.ap(), in_=osb)

    nc.compile()
    return nc


def get_nc(ppc=PPC):
    if ppc not in _CACHE:
        _CACHE[ppc] = build_nc(ppc)
    return _CACHE[ppc]


def kernel(**inputs):
    from concourse.bass_utils import run_bass_kernel_spmd

    nc = get_nc(PPC)
    names = ["desc_i", "desc_j", "kp_i", "kp_j", "F_mat"]
    arrs = {k: np.ascontiguousarray(np.asarray(inputs[k]), dtype=np.float32)
            for k in names}
    in_maps = []
    for c in range(NCORES):
        sl = slice(c * PPC, (c + 1) * PPC)
        in_maps.append({k: arrs[k][sl] for k in names})
    res = run_bass_kernel_spmd(nc, in_maps, core_ids=list(range(NCORES)))
    total_err = 0.0
    total_cnt = 0.0
    for c in range(NCORES):
        o = np.asarray(res.results[c]["out"]).reshape(-1)
        total_err += float(o[0])
        total_cnt += float(o[1])
    mean_err = np.float32(total_err) / np.float32(max(total_cnt, 1.0))
    reward = np.exp(np.float32(-mean_err))
    return np.stack([np.float32(reward), np.float32(mean_err),
                     np.float32(total_cnt)]).astype(np.float32)



# revision 21
# speedup vs baseline: 1.6762x; 1.2681x over previous
# Trainium2 Bass kernel for nn_EpipolarReward (retrieval_knn).
#
# Full computation per (batch x view-pair) p:
#   - L2-normalize desc_i (N,D) and desc_j (M,D), round to bf16
#   - gram g = di_n @ dj_n^T (bf16 x bf16 -> fp32 PSUM), both directions;
#     d2 = 2 - 2g (row/col ordering by g)
#   - top-2 per row and per column of g (DVE max8 over bf16 tiles)
#   - ratio test on squared distances: u0 < 0.64*u1 with u = relu(1 - g)
#   - mutual-nearest check + Sampson error on matched keypoints
# Sharding: P=120 pairs, embarrassingly parallel, 15 pairs per core on 8
# cores.  Each core outputs [err_sum, match_count]; the host sums partials
# and computes [exp(-mean), mean, count].
#
# v1.1: the sampson/accumulation tail runs on the Pool (GPSIMD) engine
# (arithmetic ops only; comparisons stay on DVE, which is the bottleneck
# engine) - frees ~2.5us/pair of DVE time at zero risk.
#
# Mutual matches are found transpose-free:
#   * both gram directions are computed with the same bf16 operands, so
#     g[i,j] (row direction) and gT[j,i] (col direction) are bit-identical
#     fp32 values, and their bf16 roundings agree.
#   * for each column j (partition layout j = t*128 + p) a one-pass
#     tensor_scalar builds OHc[j,:] = VJ[j] * (Cbf[j,:] == c0[j]) - for a
#     ratio-valid column the max is strictly separated so the one-hot is
#     exact with a single 1 at argmax.
#   * a small bf16 matmul scatters [1, c0, kx_hi, kx_lo, ky_hi, ky_lo] of
#     each valid column to its argmax row (fp32 PSUM sums).
#   * row i is a mutual match iff W[i]==1, scattered c0 == row max g0r[i]
#     (bit-exact), and VI[i].  A bf16 tie at the row max forces ratio==1 ->
#     VI=0, so ties cannot create false accepts.  W>=2 (two ratio-valid
#     columns argmaxing to the same row) is treated as no-match; with the
#     0.8 ratio test this is vanishingly rare.

import numpy as np

P_TOTAL, N, M, D = 120, 1024, 1024, 256
NCORES = 8
PPC = P_TOTAL // NCORES  # 15 pairs per core
NT = N // 128  # 8 row tiles
MT = M // 128  # 8 col tiles
DC = D // 128  # 2 contraction chunks

_CACHE = {}


def build_nc(ppc=PPC, repeat=1):
    import concourse.bass as bass
    import concourse.mybir as mybir
    from concourse import bacc
    from concourse.masks import make_identity
    from concourse.tile import TileContext

    f32 = mybir.dt.float32
    bf16 = mybir.dt.bfloat16
    u16 = mybir.dt.uint16
    Alu = mybir.AluOpType
    Act = mybir.ActivationFunctionType
    AX = mybir.AxisListType.X

    nc = bacc.Bacc(trn_type="TRN2", target_bir_lowering=False, debug=False,
                   num_devices=NCORES)

    di_d = nc.dram_tensor("desc_i", [ppc, N, D], f32, kind="ExternalInput")
    dj_d = nc.dram_tensor("desc_j", [ppc, M, D], f32, kind="ExternalInput")
    kpi_d = nc.dram_tensor("kp_i", [ppc, N, 2], f32, kind="ExternalInput")
    kpj_d = nc.dram_tensor("kp_j", [ppc, M, 2], f32, kind="ExternalInput")
    f_d = nc.dram_tensor("F_mat", [ppc, 3, 3], f32, kind="ExternalInput")
    out_d = nc.dram_tensor("out", [1, 2], f32, kind="ExternalOutput")

    with TileContext(nc) as tc:
        with (
            tc.tile_pool(name="const", bufs=1) as cpool,
            tc.tile_pool(name="desc", bufs=2) as dpool,
            tc.tile_pool(name="dt", bufs=2) as dtpool,
            tc.tile_pool(name="gbf", bufs=3) as gpool,
            tc.tile_pool(name="small", bufs=2) as spool,
            tc.tile_pool(name="tail", bufs=2) as tpool,
            tc.tile_pool(name="ohc", bufs=2) as ohpool,
            tc.tile_pool(name="pg", bufs=2, space="PSUM") as pgpool,
            tc.tile_pool(name="pt", bufs=1, space="PSUM") as ptpool,
            tc.tile_pool(name="pa", bufs=1, space="PSUM") as papool,
            tc.tile_pool(name="pat", bufs=1, space="PSUM") as patpool,
        ):
            # ---- constants ----
            ident = cpool.tile([128, 128], f32, tag="ident")
            make_identity(nc, ident)
            ident_bf = cpool.tile([128, 128], bf16, tag="ident_bf")
            nc.vector.tensor_copy(ident_bf, ident)
            ones_col = cpool.tile([128, 1], f32, tag="ones_col")
            nc.vector.memset(ones_col, 1.0)
            acc2 = cpool.tile([128, 2, 2], f32, tag="acc2")
            nc.vector.memset(acc2, 0.0)
            eps12 = cpool.tile([128, 1], f32, tag="eps12")
            nc.vector.memset(eps12, 1e-12)

            def norm_a(desc_ap, p, tag):
                # load + norm latency chain (squares -> reduce -> sqrt ->
                # 1/x); emitted one pair EARLY so the DVE reduce/reciprocal
                # never head-of-line block the match phase's max8 scans.
                dsb = dpool.tile([128, NT, D], f32, tag=f"dsb_{tag}")
                nc.sync.dma_start(
                    out=dsb, in_=desc_ap[p].rearrange("(t q) d -> q t d", q=128))
                ss = spool.tile([128, NT], f32, tag=f"ss_{tag}")
                scr = spool.tile([128, NT, D], f32, tag=f"scr_{tag}",
                                 name=f"scr_{tag}")
                nc.gpsimd.tensor_mul(scr, dsb, dsb)
                nc.vector.reduce_sum(ss, scr, axis=AX)
                sq = spool.tile([128, NT], f32, tag=f"sq_{tag}")
                nc.scalar.activation(sq, ss, Act.Sqrt, bias=eps12)
                rs = spool.tile([128, NT], f32, tag=f"rs_{tag}")
                nc.vector.reciprocal(rs, sq)
                return dsb, rs

            def norm_b(dsb, rs, tag):
                # scale + transpose, emitted in the original (pair-local)
                # slot so the PE/PSUM transpose traffic keeps its schedule.
                dnb = dpool.tile([128, NT, D], bf16, tag=f"dnb_{tag}")
                nc.gpsimd.tensor_mul(
                    dnb, dsb, rs.unsqueeze(2).to_broadcast([128, NT, D]))
                dt = dtpool.tile([128, DC, N], bf16, tag=f"dt_{tag}")
                for c in range(DC):
                    pt = ptpool.tile([128, 1024], bf16, tag="pt")
                    for t in range(NT):
                        nc.tensor.transpose(
                            pt[:, t * 128:(t + 1) * 128],
                            dnb[:, t, c * 128:(c + 1) * 128], ident_bf)
                    nc.scalar.copy(dt[:, c], pt)
                return dt

            def gram_scan(dta, dtb, tag, keep_tiles):
                # top-8 values per partition; optionally keep the bf16 tiles
                m8 = tpool.tile([128, NT, 8], bf16, tag=f"m8_{tag}",
                                name=f"m8_{tag}")
                tiles = []
                for t in range(NT):
                    gb = gpool.tile([128, M], bf16, tag=f"gb_{tag}",
                                    name=f"gb_{tag}",
                                    bufs=(NT + 1 if keep_tiles else 3))
                    pg = pgpool.tile([128, 1024], f32, tag="pg", name="pg")
                    for jc in range(2):
                        for c in range(DC):
                            nc.tensor.matmul(
                                pg[:, jc * 512:(jc + 1) * 512],
                                lhsT=dta[:, c, t * 128:(t + 1) * 128],
                                rhs=dtb[:, c, jc * 512:(jc + 1) * 512],
                                start=(c == 0), stop=(c == DC - 1))
                    nc.scalar.copy(gb, pg)
                    nc.vector.max(m8[:, t], gb)
                    if keep_tiles:
                        tiles.append(gb)
                return m8, tiles

            npairs = ppc * repeat
            na = (norm_a(di_d.ap(), 0, "i"), norm_a(dj_d.ap(), 0, "j"))
            for pi in range(npairs):
                p = pi % ppc
                dti = norm_b(*na[0], "i")
                dtj = norm_b(*na[1], "j")
                m8r, _ = gram_scan(dti, dtj, "r", keep_tiles=False)
                m8c, ctiles = gram_scan(dtj, dti, "c", keep_tiles=True)
                if pi + 1 < npairs:
                    pn = (pi + 1) % ppc
                    na = (norm_a(di_d.ap(), pn, "i"), norm_a(dj_d.ap(), pn, "j"))

                kpi = tpool.tile([128, NT, 2], f32, tag="kpi")
                nc.sync.dma_start(
                    out=kpi, in_=kpi_d.ap()[p].rearrange("(t q) c -> q t c", q=128))
                kpj = tpool.tile([128, MT, 2], f32, tag="kpj")
                nc.sync.dma_start(
                    out=kpj, in_=kpj_d.ap()[p].rearrange("(t q) c -> q t c", q=128))
                frow = tpool.tile([128, 9], f32, tag="frow")
                nc.sync.dma_start(
                    out=frow,
                    in_=f_d.ap()[p].rearrange("a b -> (a b)").partition_broadcast(128))

                # u = relu(1 - g) (prop. to d2) from top-2 row/col values
                dd = tpool.tile([128, 32], f32, tag="dd")
                nc.scalar.copy(dd[:, 0:8], m8r[:, :, 0])
                nc.scalar.copy(dd[:, 8:16], m8r[:, :, 1])
                nc.scalar.copy(dd[:, 16:24], m8c[:, :, 0])
                nc.scalar.copy(dd[:, 24:32], m8c[:, :, 1])
                uu = tpool.tile([128, 32], f32, tag="uu")
                nc.vector.tensor_scalar(uu, dd, -1.0, 1.0, op0=Alu.mult, op1=Alu.add)
                nc.vector.tensor_scalar_max(uu, uu, 0.0)
                # valid = u0 < 0.64*u1   (ratio(dist) < 0.8)
                vthr = tpool.tile([128, 16], f32, tag="vthr")
                nc.vector.tensor_scalar(vthr[:, 0:8], uu[:, 8:16], 0.64, None,
                                        op0=Alu.mult)
                nc.vector.tensor_scalar(vthr[:, 8:16], uu[:, 24:32], 0.64, None,
                                        op0=Alu.mult)
                vi = tpool.tile([128, 8], f32, tag="vi")
                nc.vector.tensor_tensor(vi, uu[:, 0:8], vthr[:, 0:8], Alu.is_lt)
                vj = tpool.tile([128, 8], f32, tag="vj")
                nc.vector.tensor_tensor(vj, uu[:, 16:24], vthr[:, 8:16], Alu.is_lt)

                # scatter table: [1, c0, kxh, kxl, kyh, kyl, 0, 0] per column
                ct = tpool.tile([128, MT, 8], bf16, tag="ct")
                nc.vector.memset(ct[:, :, 0], 1.0)
                nc.vector.tensor_copy(ct[:, :, 1], m8c[:, :, 0])
                tmh = tpool.tile([128, MT], f32, tag="tmh")
                tml = tpool.tile([128, MT], f32, tag="tml")
                for comp in range(2):
                    hi_c, lo_c = 2 + 2 * comp, 3 + 2 * comp
                    nc.vector.tensor_copy(ct[:, :, hi_c], kpj[:, :, comp])
                    nc.vector.tensor_copy(tmh, ct[:, :, hi_c])
                    nc.vector.tensor_sub(tml, kpj[:, :, comp], tmh)
                    nc.vector.tensor_copy(ct[:, :, lo_c], tml)
                nc.vector.memset(ct[:, :, 6], 0.0)
                nc.vector.memset(ct[:, :, 7], 0.0)

                pa = [papool.tile([8, 512], f32, tag=f"pa{ic}", name=f"pa{ic}")
                      for ic in range(2)]
                for t in range(MT):
                    # masked one-hot: VJ[j] * (Cbf[j,:] == c0[j])
                    oh = ohpool.tile([128, N], bf16, tag="oh", name="oh")
                    nc.vector.tensor_scalar(
                        oh, ctiles[t], dd[:, 16 + t:17 + t], vj[:, t:t + 1],
                        op0=Alu.is_equal, op1=Alu.mult)
                    for ic in range(2):
                        nc.tensor.matmul(
                            pa[ic], lhsT=ct[:, t],
                            rhs=oh[:, ic * 512:(ic + 1) * 512],
                            start=(t == 0), stop=(t == MT - 1))
                asb = tpool.tile([8, N], f32, tag="asb")
                for ic in range(2):
                    nc.scalar.copy(asb[:, ic * 512:(ic + 1) * 512], pa[ic])
                pat = patpool.tile([128, 64], f32, tag="pat")
                for t in range(NT):
                    nc.tensor.transpose(pat[:, t * 8:(t + 1) * 8],
                                        asb[:, t * 128:(t + 1) * 128], ident[:8, :8])
                at = tpool.tile([128, NT, 8], f32, tag="at")
                nc.scalar.copy(at.rearrange("p a b -> p (a b)"), pat)

                # mutual = vi & (W == 1) & (scattered c0 == row max)
                g0f = dd[:, 0:8]
                e1 = tpool.tile([128, 8], f32, tag="e1")
                nc.vector.tensor_scalar(e1, at[:, :, 0], 1.0, None, op0=Alu.is_equal)
                e2 = tpool.tile([128, 8], f32, tag="e2")
                nc.vector.tensor_tensor(e2, at[:, :, 1], g0f, Alu.is_equal)
                mu = tpool.tile([128, 8], f32, tag="mu")
                nc.gpsimd.tensor_mul(mu, vi, e1)
                nc.gpsimd.tensor_mul(mu, mu, e2)

                kx = tpool.tile([128, 8], f32, tag="kx")
                nc.gpsimd.tensor_add(kx, at[:, :, 2], at[:, :, 3])
                ky = tpool.tile([128, 8], f32, tag="ky")
                nc.gpsimd.tensor_add(ky, at[:, :, 4], at[:, :, 5])

                # sampson: xi = (kpi_x, kpi_y, 1), xj = (kx, ky, 1)
                # l_j[k] = F[k,0]*xi_x + F[k,1]*xi_y + F[k,2]
                # l_i[k] = F[0,k]*xj_x + F[1,k]*xj_y + F[2,k]
                ta = tpool.tile([128, 8], f32, tag="ta")
                tb = tpool.tile([128, 8], f32, tag="tb")
                lj = [tpool.tile([128, 8], f32, tag=f"lj{k}", name=f"lj{k}")
                      for k in range(3)]
                for k in range(3):
                    nc.gpsimd.tensor_scalar_mul(ta, kpi[:, :, 0],
                                                frow[:, 3 * k:3 * k + 1])
                    nc.gpsimd.tensor_scalar_mul(tb, kpi[:, :, 1],
                                                frow[:, 3 * k + 1:3 * k + 2])
                    nc.gpsimd.tensor_add(lj[k], ta, tb)
                    nc.gpsimd.tensor_scalar_add(lj[k], lj[k],
                                                frow[:, 3 * k + 2:3 * k + 3])
                li = [tpool.tile([128, 8], f32, tag=f"li{k}", name=f"li{k}")
                      for k in range(2)]
                for k in range(2):
                    nc.gpsimd.tensor_scalar_mul(ta, kx, frow[:, k:k + 1])
                    nc.gpsimd.tensor_scalar_mul(tb, ky, frow[:, 3 + k:4 + k])
                    nc.gpsimd.tensor_add(li[k], ta, tb)
                    nc.gpsimd.tensor_scalar_add(li[k], li[k], frow[:, 6 + k:7 + k])
                s = tpool.tile([128, 8], f32, tag="s")
                nc.gpsimd.tensor_mul(s, kx, lj[0])
                nc.gpsimd.tensor_mul(ta, ky, lj[1])
                nc.gpsimd.tensor_add(s, s, ta)
                nc.gpsimd.tensor_add(s, s, lj[2])
                num = tpool.tile([128, 8], f32, tag="num")
                nc.gpsimd.tensor_mul(num, s, s)
                den = tpool.tile([128, 8], f32, tag="den")
                nc.gpsimd.tensor_mul(den, lj[0], lj[0])
                nc.gpsimd.tensor_mul(ta, lj[1], lj[1])
                nc.gpsimd.tensor_add(den, den, ta)
                nc.gpsimd.tensor_mul(ta, li[0], li[0])
                nc.gpsimd.tensor_add(den, den, ta)
                nc.gpsimd.tensor_mul(ta, li[1], li[1])
                nc.gpsimd.tensor_add(den, den, ta)
                nc.gpsimd.tensor_scalar_add(den, den, 1e-6)
                rden = tpool.tile([128, 8], f32, tag="rden")
                nc.vector.reciprocal(rden, den)
                err = tpool.tile([128, 8], f32, tag="err")
                nc.gpsimd.tensor_mul(err, num, rden)
                nc.gpsimd.tensor_mul(err, err, mu)
                rsum = tpool.tile([128, 2], f32, tag="rsum")
                t4 = tpool.tile([128, 2, 4], f32, tag="t4")
                t2 = tpool.tile([128, 2, 2], f32, tag="t2")
                for k, srcx in enumerate((err, mu)):
                    nc.gpsimd.tensor_add(t4[:, k], srcx[:, 0:4], srcx[:, 4:8])
                    nc.gpsimd.tensor_add(t2[:, k], t4[:, k, 0:2], t4[:, k, 2:4])
                    nc.gpsimd.tensor_add(rsum[:, k:k + 1], t2[:, k, 0:1],
                                         t2[:, k, 1:2])
                nc.gpsimd.tensor_add(acc2[:, p % 2], acc2[:, p % 2], rsum)

            # final cross-partition reduce: out[0, :] = sum_p acc[p, :]
            acc = cpool.tile([128, 2], f32, tag="accf")
            nc.vector.tensor_add(acc, acc2[:, 0], acc2[:, 1])
            pf = papool.tile([1, 2], f32, tag="pa0")
            nc.tensor.matmul(pf, lhsT=ones_col, rhs=acc, start=True, stop=True)
            osb = cpool.tile([1, 2], f32, tag="osb")
            nc.vector.tensor_copy(osb, pf)
            nc.sync.dma_start(out=out_d.ap(), in_=osb)

    nc.compile()
    return nc


def get_nc(ppc=PPC):
    if ppc not in _CACHE:
        _CACHE[ppc] = build_nc(ppc)
    return _CACHE[ppc]


def kernel(**inputs):
    from concourse.bass_utils import run_bass_kernel_spmd

    nc = get_nc(PPC)
    names = ["desc_i", "desc_j", "kp_i", "kp_j", "F_mat"]
    arrs = {k: np.ascontiguousarray(np.asarray(inputs[k]), dtype=np.float32)
            for k in names}
    in_maps = []
    for c in range(NCORES):
        sl = slice(c * PPC, (c + 1) * PPC)
        in_maps.append({k: arrs[k][sl] for k in names})
    res = run_bass_kernel_spmd(nc, in_maps, core_ids=list(range(NCORES)))
    total_err = 0.0
    total_cnt = 0.0
    for c in range(NCORES):
        o = np.asarray(res.results[c]["out"]).reshape(-1)
        total_err += float(o[0])
        total_cnt += float(o[1])
    mean_err = np.float32(total_err) / np.float32(max(total_cnt, 1.0))
    reward = np.exp(np.float32(-mean_err))
    return np.stack([np.float32(reward), np.float32(mean_err),
                     np.float32(total_cnt)]).astype(np.float32)

